# revision 21
# baseline (speedup 1.0000x reference)
"""Bayesian spectral convolution (FNO-style) Trainium2 kernel.

Math (verified vs reference to 3e-7 rel err):
  out[b,o,h,w] = sum_{kh<16,kw<9} R[b,o,kh,kw] * (Ch[kh,h]*a[kw,w] - Sh[kh,h]*bm[kw,w]) / (H*W)
  R[b,o,kh,kw] = sum_c XcRe[b,c,kh,kw]*w_re[o,c,kh,kw] + XcIm'[b,c,kh,kw]*w_im[o,c,kh,kw]
  XcRe = CC - SS ; XcIm' = SC + CS   (C*/S* = cos/sin DFT partial transforms of x)
  w_re/w_im = reparameterized Bayesian weight sample with fixed PRNGKey(2)/(3) noise.

All transforms are small dense matmuls (no FFT needed). Sharding: data-parallel
over batch (16 -> 2 per core x 8 cores); spectral weights replicated.
The second output (broadcast aleatoric scalar) is computed on host (it only
depends on the small logvar inputs).
"""

import os

import numpy as np

B, CIN, COUT, H, W = 16, 64, 64, 128, 128
KH, KW = 16, 9
NCORES = 8
BLOC = B // NCORES          # 2 batch per core
BC = BLOC * CIN             # 128 (b,c) pairs per core
BO = BLOC * COUT            # 128 (b,o) pairs per core
NMODE = KH * KW             # 144

_CACHE = {}


def _consts():
    """Host-precomputed DFT basis matrices (fp32)."""
    if "consts" in _CACHE:
        return _CACHE["consts"]
    h = np.arange(H)
    kh = np.arange(KH)
    kw = np.arange(KW)
    ang_h = 2.0 * np.pi * np.outer(kh, h) / H          # (16, 128)
    ang_w = 2.0 * np.pi * np.outer(kw, h) / W          # (9, 128)
    FhC, FhS = np.cos(ang_h), np.sin(ang_h)
    FwC, FwS = np.cos(ang_w), np.sin(ang_w)
    # forward H-transform, stationary rhs [h, 32]
    fht = np.concatenate([FhC.T, FhS.T], axis=1).astype(np.float32)      # (128, 32)
    # forward W-transform rhs [w, 18]
    fwt = np.concatenate([FwC.T, FwS.T], axis=1).astype(np.float32)      # (128, 18)
    # inverse W basis stacked on K=64 (32-aligned blocks): rows 0:9 = a,
    # rows 32:41 = bm, rest zero (padding keeps partition bases 32-aligned)
    scale2 = np.where(kw[:, None] == 0, 1.0, 2.0)
    a_mat = scale2 * FwC
    b_mat = np.where(kw[:, None] == 0, 0.0, 2.0) * FwS
    ab = np.zeros((64, W), np.float32)
    ab[0:KW] = a_mat
    ab[32:32 + KW] = b_mat
    # inverse H basis: rows 0:16 = Ch/(H*W), 16:32 = -Sh/(H*W); replicated 4x
    # on partitions so stage-5 row-group tiles find weights at base 32*i
    hinv1 = np.concatenate([FhC, -FhS], axis=0) / (H * W)
    hinv = np.tile(hinv1, (4, 1)).astype(np.float32)                     # (128, 128)
    _CACHE["consts"] = (fht, fwt, ab, hinv)
    return _CACHE["consts"]


def _eps():
    """Deterministic reparameterization noise (fixed keys in the source model)."""
    if "eps" in _CACHE:
        return _CACHE["eps"]
    import jax

    cpu = jax.devices("cpu")[0]
    with jax.default_device(cpu):
        er = np.asarray(jax.random.normal(jax.random.PRNGKey(2), (COUT, CIN, KH, KW)))
        ei = np.asarray(jax.random.normal(jax.random.PRNGKey(3), (COUT, CIN, KH, KW)))
    _CACHE["eps"] = (er.astype(np.float32), ei.astype(np.float32))
    return _CACHE["eps"]


def _build_bass():
    if "nc" in _CACHE:
        return _CACHE["nc"]
    import concourse.mybir as mybir
    from concourse import bacc
    from concourse.masks import make_identity
    from concourse.tile import TileContext

    f32 = mybir.dt.float32
    nc = bacc.Bacc()

    x_d = nc.declare_dram_parameter("x", [BC, H, W], f32, isOutput=False)
    ws_d = nc.declare_dram_parameter("wstack", [2 * CIN, NMODE * COUT], f32, isOutput=False)
    fht_d = nc.declare_dram_parameter("fht", [H, 32], f32, isOutput=False)
    fwt_d = nc.declare_dram_parameter("fwt", [W, 18], f32, isOutput=False)
    ab_d = nc.declare_dram_parameter("ab", [64, W], f32, isOutput=False)
    hinv_d = nc.declare_dram_parameter("hinv", [128, H], f32, isOutput=False)
    out_d = nc.declare_dram_parameter("out", [BO, H, W], f32, isOutput=True)

    GN = 16                      # (b,c) pairs per stage-1 DMA group
    NGB = CIN // GN              # 4 groups per batch half

    with TileContext(nc) as tc:
        with (
            tc.tile_pool(name="consts", bufs=1) as consts,
            tc.tile_pool(name="xin", bufs=8) as xin,
            tc.tile_pool(name="ypool", bufs=2) as ypool,
            tc.tile_pool(name="work", bufs=2) as work,
            tc.tile_pool(name="s2p", bufs=4) as s2pool,
            tc.tile_pool(name="ustack", bufs=4) as ust,
            tc.tile_pool(name="ostage", bufs=2) as ost,
            tc.tile_pool(name="psA", bufs=3, space="PSUM") as psA,
            tc.tile_pool(name="psB", bufs=5, space="PSUM") as psB,
        ):
            # ---- constants in SBUF (small ones first; wstack streams after
            #      the first batch-half's x tiles so PE can start early) ----
            fht = consts.tile([H, 32], f32)
            nc.sync.dma_start(out=fht[:], in_=fht_d[:])
            fwt = consts.tile([W, 18], f32)
            nc.sync.dma_start(out=fwt[:], in_=fwt_d[:])
            ab = consts.tile([64, W], f32)
            nc.sync.dma_start(out=ab[:], in_=ab_d[:])
            hinv = consts.tile([128, H], f32)
            nc.sync.dma_start(out=hinv[:], in_=hinv_d[:])
            ident = consts.tile([128, 128], f32)
            make_identity(nc, ident)

            # preload b0's x tiles; wstack streams concurrently on the
            # SWDGE (gpsimd) queue so x-b1 needn't wait behind it on SP
            pre_xg = []
            for gg in range(2 * NGB):
                xg = xin.tile([128, GN * W], f32, tag="xg")
                nc.sync.dma_start(
                    out=xg[:].rearrange("h (c w) -> h c w", w=W),
                    in_=x_d[gg * GN:(gg + 1) * GN].rearrange("c h w -> h c w"),
                )
                pre_xg.append(xg)

            wstack = consts.tile([2 * CIN, NMODE * COUT], f32)
            wch = NMODE * COUT // 8
            for i in range(8):
                nc.gpsimd.dma_start(
                    out=wstack[:, i * wch:(i + 1) * wch],
                    in_=ws_d[:, i * wch:(i + 1) * wch],
                )

            # BigLhsT tiles pre-zeroed during the DMA head; per-b copies
            # overwrite only the R^T blocks (rows 0:9 and 32:41)
            big_tiles = []
            for b in range(BLOC):
                bigb = work.tile([64, COUT * 32], f32, tag="big")
                nc.gpsimd.memset(bigb[:], 0.0)
                big_tiles.append(bigb)

            # ---- forward pass for both batch halves (S1+S2+combine),
            #      then inverse pass (mix+transpose+S4/S5+store).  This keeps
            #      PE busy with b1's forward while wstack streams in.
            for b in range(BLOC):
                yb = ypool.tile([128, CIN * 32], f32, tag="y")
                for g in range(NGB):
                    xg = pre_xg[b * NGB + g]
                    p1 = psA.tile([128, GN * 32], f32, tag="pa")
                    for i in range(GN):
                        nc.tensor.matmul(
                            p1[:, i * 32:(i + 1) * 32],
                            lhsT=xg[:, i * W:(i + 1) * W],
                            rhs=fht[:],
                            start=True, stop=True,
                        )
                    if g % 2 == 0:
                        nc.vector.tensor_copy(yb[:, g * 512:(g + 1) * 512], p1[:])
                    else:
                        nc.scalar.copy(yb[:, g * 512:(g + 1) * 512], p1[:])

                yv = yb[:].rearrange("w (c j) -> w c j", j=32)
                xmixb = work.tile([2 * CIN, NMODE], f32, tag="xmix")
                sb = []
                for p in range(2):
                    p2 = psB.tile([CIN, KH * 18], f32, tag="pb")
                    for k in range(KH):
                        nc.tensor.matmul(
                            p2[:, k * 18:(k + 1) * 18],
                            lhsT=yv[:, :, p * KH + k],
                            rhs=fwt[:],
                            start=True, stop=True,
                        )
                    s2 = s2pool.tile([CIN, KH * 18], f32, tag="s2")
                    nc.vector.tensor_copy(s2[:], p2[:])
                    sb.append(s2)
                o_re = xmixb[0:CIN, :].rearrange("c (k j) -> c k j", j=KW)
                o_im = xmixb[CIN:2 * CIN, :].rearrange("c (k j) -> c k j", j=KW)
                cc = sb[0][:].rearrange("c (k q) -> c k q", q=18)
                ss = sb[1][:].rearrange("c (k q) -> c k q", q=18)
                nc.vector.tensor_sub(o_re, cc[:, :, 0:KW], ss[:, :, 9:9 + KW])
                nc.vector.tensor_add(o_im, ss[:, :, 0:KW], cc[:, :, 9:9 + KW])

                # mix: modes m and m+72 packed into PE col-groups
                psc = psB.tile([128, NMODE // 2], f32, tag="pb")
                for p in range(NMODE // 2):
                    nc.tensor.matmul(
                        psc[0:COUT, p:p + 1],
                        lhsT=wstack[:, p * 2 * COUT:p * 2 * COUT + COUT],
                        rhs=xmixb[:, p:p + 1],
                        start=True, stop=True,
                        tile_position=(0, 0),
                    )
                    nc.tensor.matmul(
                        psc[COUT:128, p:p + 1],
                        lhsT=wstack[:, p * 2 * COUT + COUT:(p + 1) * 2 * COUT],
                        rhs=xmixb[:, p + 72:p + 73],
                        start=True, stop=True,
                        tile_position=(0, 64),
                    )
                rcb = work.tile([128, NMODE // 2], f32, tag="rc")
                nc.scalar.copy(rcb[:], psc[:])

                # transpose R -> rtb [kw=9, (kh, o)]
                rtb = work.tile([KW, KH * COUT], f32, tag="rt")
                for half in range(2):
                    pt = psB.tile([KW, 8 * COUT], f32, tag="pb")
                    r0 = half * COUT
                    for kk in range(8):
                        nc.tensor.transpose(
                            pt[:, kk * COUT:(kk + 1) * COUT],
                            rcb[r0:r0 + COUT, kk * KW:(kk + 1) * KW],
                            ident[r0:r0 + COUT, r0:r0 + COUT],
                        )
                    nc.vector.tensor_copy(
                        rtb[:, half * 512:(half + 1) * 512], pt[:])

                # BigLhsT [64, (o, 32)]: R^T at rows 0:9 (UA) and 32:41 (UB)
                bigb = big_tiles[b]
                bigv = bigb[:].rearrange("p (o j) -> p o j", o=COUT)
                rtv = rtb[:].rearrange("p (k o) -> p o k", k=KH)
                nc.vector.tensor_copy(bigv[0:KW, :, 0:KH], rtv[:, :, :])
                nc.vector.tensor_copy(bigv[32:32 + KW, :, KH:2 * KH], rtv[:, :, :])

                # stages 4+5 per group of 8 o; stage 4 packs 4 o per matmul
                for og in range(COUT // 8):
                    u_tiles = []
                    for q in range(2):
                        o0 = og * 8 + q * 4
                        p4 = psB.tile([128, W], f32, tag="pb")
                        nc.tensor.matmul(
                            p4[:],
                            lhsT=bigb[:, o0 * 32:(o0 + 4) * 32],
                            rhs=ab[:],
                            start=True, stop=True,
                        )
                        u = ust.tile([32, 4 * W], f32, tag="u")
                        for i in range(4):
                            if i % 2:
                                nc.scalar.copy(
                                    u[:, i * W:(i + 1) * W],
                                    p4[32 * i:32 * (i + 1), :])
                            else:
                                nc.vector.tensor_copy(
                                    u[:, i * W:(i + 1) * W],
                                    p4[32 * i:32 * (i + 1), :])
                        u_tiles.append(u)
                    outb = ost.tile([128, 8 * W], f32, tag="outb")
                    for q in range(2):
                        p5 = psA.tile([128, 4 * W], f32, tag="pa")
                        nc.tensor.matmul(
                            p5[:],
                            lhsT=hinv[0:32, :],
                            rhs=u_tiles[q][:],
                            start=True, stop=True,
                        )
                        if q % 2 == 0:
                            nc.vector.tensor_copy(
                                outb[:, q * 4 * W:(q + 1) * 4 * W], p5[:])
                        else:
                            nc.scalar.copy(
                                outb[:, q * 4 * W:(q + 1) * 4 * W], p5[:])
                    bo0 = b * COUT + og * 8
                    nc.sync.dma_start(
                        out=out_d[bo0:bo0 + 8].rearrange("c h w -> h c w"),
                        in_=outb[:].rearrange("h (c w) -> h c w", w=W),
                    )

    nc.finalize()
    _CACHE["nc"] = nc
    return nc


def kernel(x, weight_mean, weight_logvar, weight_imag_mean, weight_imag_logvar):
    from concourse.bass_utils import run_bass_kernel_spmd

    x = np.ascontiguousarray(np.asarray(x, dtype=np.float32))
    wm = np.asarray(weight_mean, dtype=np.float32)
    wlv = np.asarray(weight_logvar, dtype=np.float32)
    wim = np.asarray(weight_imag_mean, dtype=np.float32)
    wilv = np.asarray(weight_imag_logvar, dtype=np.float32)

    # ---- host: Bayesian weight sample + aleatoric scalar (fp32 to match ref) ----
    eps_r, eps_i = _eps()
    real_std = np.exp(np.float32(0.5) * wlv)
    imag_std = np.exp(np.float32(0.5) * wilv)
    w_re = wm + real_std * eps_r
    w_im = wim + imag_std * eps_i
    # mirror the reference's jnp f32 ops exactly (accumulation order matters)
    import jax
    import jax.numpy as jnp

    with jax.default_device(jax.devices("cpu")[0]):
        rs = jnp.exp(0.5 * jnp.asarray(wlv))
        is_ = jnp.exp(0.5 * jnp.asarray(wilv))
        aleatoric = np.asarray(jnp.sqrt(jnp.mean(rs**2 + is_**2)), dtype=np.float32)

    # Wstack rows 0:64 = w_re[o,c,m], 64:128 = w_im.  Columns hold mode PAIRS
    # (p, p+72): col block p*128 = [64 cols mode p | 64 cols mode p+72]
    def _pack(wt):
        t = wt.transpose(1, 2, 3, 0).reshape(CIN, 2, NMODE // 2, COUT)
        return t.transpose(0, 2, 1, 3).reshape(CIN, NMODE * COUT)

    ws = np.empty((2 * CIN, NMODE * COUT), np.float32)
    ws[0:CIN] = _pack(w_re)
    ws[CIN:] = _pack(w_im)

    fht, fwt, ab, hinv = _consts()
    nc = _build_bass()

    in_maps = []
    for c in range(NCORES):
        xl = x[c * BLOC:(c + 1) * BLOC].reshape(BC, H, W)
        in_maps.append({
            "x": xl, "wstack": ws, "fht": fht, "fwt": fwt,
            "ab": ab, "hinv": hinv,
        })

    trace = bool(int(os.environ.get("KERNEL_TRACE", "0")))
    res = run_bass_kernel_spmd(nc, in_maps, list(range(NCORES)), trace=trace)
    _CACHE["last_result"] = res

    out = np.empty((B, COUT, H, W), np.float32)
    for c in range(NCORES):
        out[c * BLOC:(c + 1) * BLOC] = res.results[c]["out"].reshape(BLOC, COUT, H, W)

    alea_full = np.broadcast_to(aleatoric, out.shape)
    return out, alea_full
